# revision 29
# baseline (speedup 1.0000x reference)
"""Distributed single-head attention on 8 TRN2 NeuronCores — zero-collective.

Math (matches the reference):
    q = z @ Wq; k = z @ Wk; v = z @ Wv
    out = softmax(q k^T) * DK**-0.5 @ v

Key idea: every core receives the FULL z (inputs are full-size anyway), so
k and v never need to be materialized or all-gathered.  Using associativity:
    S_r  = q_r k^T = (z_r Wq) Wk^T z^T      -> B^T = Wk q_r^T, S^T = z B^T
    out_r = P_r v  = (P_r z) Wv             -> C^T = z^T-accum of P^T, out = C^T^T Wv
Per-core FLOPs are identical to the gather-based flash schedule (736 unit
matmuls), but there are NO collectives: no skew-absorbing barrier, no
serialized CC stream, no PE stall waiting for gathered K/V (the baseline
idled the PE 38us there, which also re-throttled the HAM clock gate).

Sharding: core c processes rows [512c, 512c+512).  Host ships z rolled so
each core's own block is first: zT_roll (d-major, for S) and zn_roll
(seq-major, for C).  A j-tile index in the kernel is the global row
(c*512 + 128j) mod 4096 — a pure permutation, harmless under the j-sums.

Phases (all PE-dense, back-to-back):
    q^T (64 MM) -> B^T (64) -> S^T/exp/rowsum (256+32) -> C^T (256) -> out (64)
DMA: ~22MB of params per core against ~255-358GB/s aggregate read BW, so
strict global need order: (wq scalar || zT sync) for q, WkT then zT tails
on sync, zn prefetch + Wv only from S-phase start.  PSUM: 8 banks for
projections, 2+1 for S/rowsum, 8 for C^T, 8 for out — sequential scopes.

Precision: fp16 z/W/q/B + f32 PSUM keeps logits to ~6e-3 abs err; exp and
C^T in bf16 (range: logits can reach ~70 pre-shift, so exp(S-40) can hit
e^30 — fp16 would overflow, bf16 is safe).  End-to-end rel err ~3e-3.
"""

import numpy as np

SEQ, D, DK, DV = 4096, 1024, 1024, 1024
NCORES = 8
ROWS = SEQ // NCORES            # 512 rows per core
DT = D // 128                   # 8 contraction tiles (input dim)
MT = DK // 128                  # 8 dk tiles
ST = ROWS // 128                # 4 local seq tiles
JT = SEQ // 128                 # 32 global seq tiles
SHIFT = 40.0                    # constant logit shift (softmax-invariant)
SCALE = DK ** -0.5


def _build():
    import concourse.mybir as mybir
    import concourse.tile as tile
    from concourse import bacc

    F32 = mybir.dt.float32
    F16 = mybir.dt.float16
    BF16 = mybir.dt.bfloat16
    Exp = mybir.ActivationFunctionType.Exp

    nc = bacc.Bacc("TRN2", target_bir_lowering=False, debug=False, num_devices=NCORES)
    d_zT = nc.declare_dram_parameter("zT", [D, SEQ], F16, isOutput=False)
    d_zn = nc.declare_dram_parameter("zn", [SEQ, D], F16, isOutput=False)
    d_wq = nc.declare_dram_parameter("Wq", [D, DK], F16, isOutput=False)
    d_wkt = nc.declare_dram_parameter("WkT", [DK, D], F16, isOutput=False)
    d_wv = nc.declare_dram_parameter("Wv", [D, DV], F16, isOutput=False)
    d_out = nc.declare_dram_parameter("out", [ROWS, DV], F32, isOutput=True)

    with tile.TileContext(nc) as tc:
        with (
            tc.tile_pool(name="dram", bufs=1, space="DRAM") as dram,
            tc.tile_pool(name="misc", bufs=1) as misc,
            tc.tile_pool(name="zt", bufs=1) as ztp,
            tc.tile_pool(name="expp", bufs=1) as expp,
            tc.tile_pool(name="wvp", bufs=1) as wvp,
            tc.tile_pool(name="qb", bufs=1) as qbp,
            tc.tile_pool(name="outp", bufs=4) as outp,
        ):
            # constants: full-width ones for the PE rowsum, exp bias
            ones128 = misc.tile([128, 128], BF16)
            nc.vector.memset(ones128[:], 1.0)
            bias_sb = misc.tile([128, 1], F32)
            nc.vector.memset(bias_sb[:], -SHIFT)
            warm_sb = misc.tile([128, 1], F32)

            # ---- resident loads ------------------------------------------
            # sync ring, strict need order: (wq_t, zT jchunk0_t) pairs so the
            # t-outer q projection starts after one pair; then WkT for B^T;
            # then the rest of zT for the S phase.
            ztv = d_zT.rearrange("(t p) n -> p t n", p=128)
            wqv = d_wq.rearrange("(t p) m -> p t m", p=128)
            wktv = d_wkt.rearrange("(t p) m -> p t m", p=128)
            wvv = d_wv.rearrange("(t p) m -> p t m", p=128)

            zt_sb = ztp.tile([128, DT, SEQ], F16)
            # Wv loads late (gpsimd ring, at S-phase start): per-core
            # AGGREGATE DMA read BW is only ~255GB/s across all rings, so
            # anything loading during the projection phase steals bandwidth
            # from the critical (wq, zT, WkT) stream
            wv_sb = wvp.tile([128, DT, DV], F16)

            B_sb = qbp.tile([128, MT, ROWS], F16)
            expS = expp.tile([128, JT, ROWS], BF16)

            with (
                tc.tile_pool(name="wqk", bufs=1) as wqk,
                tc.tile_pool(name="ps_proj", bufs=8, space="PSUM") as psp,
            ):
                # q-phase feed: while both rings pull, each gets only
                # ~half the ~255-358GB/s per-core aggregate — less than the
                # q pass's per-ring appetite if wq rides one ring alone.
                # So wq interleaves across BOTH rings by t-parity, the sync
                # ring carries only the [0:512] zT halves q actually reads,
                # and everything q/B doesn't need yet (zT [512:1024] halves,
                # jchunk1-3) loads after WkT.  Every operand then arrives
                # >=1.5us before its consuming matmul group.
                wq_sb = [wqk.tile([128, DK], F16, name=f"wq{t}")
                         for t in range(DT)]
                wkt_sb = [wqk.tile([128, D], F16, name=f"wkt{t}")
                          for t in range(DT)]
                # scalar ring: even-t wq (first one split so MM#0 starts asap)
                nc.scalar.dma_start(wq_sb[0][:, 0:128], wqv[:, 0, 0:128])
                nc.scalar.dma_start(wq_sb[0][:, 128:DK], wqv[:, 0, 128:DK])
                for t in (2, 4, 6):
                    nc.scalar.dma_start(wq_sb[t][:], wqv[:, t, :])
                # sync ring: zT own-halves with odd-t wq woven in need order
                for t in range(DT):
                    nc.sync.dma_start(zt_sb[:, t, 0:512], ztv[:, t, 0:512])
                    if t % 2 == 0:
                        nc.sync.dma_start(wq_sb[t + 1][:], wqv[:, t + 1, :])
                for t in range(DT):
                    nc.sync.dma_start(wkt_sb[t][:], wktv[:, t, :])
                for t in range(DT):
                    nc.sync.dma_start(zt_sb[:, t, 512:1024],
                                      ztv[:, t, 512:1024])
                # touch Exp only AFTER the scalar ring's wq issues: the
                # 1.3us ACT table load was delaying the first q operands;
                # it just needs to finish before the first real exp (~46us)
                nc.scalar.activation(warm_sb[:], bias_sb[:], Exp,
                                     bias=bias_sb[:], scale=1.0)
                # zT tails at per-jchunk granularity so S j>=8 never waits
                # on bytes it doesn't need yet
                for c in range(1, 4):
                    for t in range(DT):
                        nc.sync.dma_start(zt_sb[:, t, c * 1024:(c + 1) * 1024],
                                          ztv[:, t, c * 1024:(c + 1) * 1024])

                # q^T[dk, r] = sum_t Wq[t-rows, dk-slice]^T zT[t-rows, own r]
                q_sb = wqk.tile([128, MT, ROWS], F16, name="qsb")
                psq = [psp.tile([128, ROWS], F32, tag="psp", name=f"psq{m}")
                       for m in range(MT)]
                for t in range(DT):
                    for m in range(MT):
                        nc.tensor.matmul(
                            psq[m][:], wq_sb[t][:, m * 128:(m + 1) * 128],
                            zt_sb[:, t, 0:ROWS],
                            start=(t == 0), stop=(t == DT - 1))
                        if t == DT - 1:
                            # copy each bank the moment it stops, alternating
                            # engines, so bank reuse in B never waits a burst
                            eng = (nc.vector.tensor_copy if m % 2 == 0
                                   else nc.scalar.copy)
                            eng(q_sb[:, m, :], psq[m][:])

                # B^T[d, r] = sum_dk Wk[d-slice, dk]^T q^T -> lhsT = WkT tiles
                # t-outer so group t only needs wkt_t (arriving just-in-time
                # behind the zT jchunk0 stream)
                psb = [psp.tile([128, ROWS], F32, tag="psp", name=f"psb{m}")
                       for m in range(MT)]
                for t in range(MT):
                    for m in range(DT):
                        nc.tensor.matmul(
                            psb[m][:], wkt_sb[t][:, m * 128:(m + 1) * 128],
                            q_sb[:, t, :],
                            start=(t == 0), stop=(t == MT - 1))
                        if t == MT - 1:
                            eng = (nc.vector.tensor_copy if m % 2 == 0
                                   else nc.scalar.copy)
                            eng(B_sb[:, m, :], psb[m][:])

            # ---------------- S phase -------------------------------------
            # S^T[j, r] = sum_t zT[t, j-slice]^T B^T[t, r]; exp on ACT with
            # the -SHIFT bias; rowsum via ones-matmul accumulated across all
            # j into one persistent PSUM bank.
            znv = d_zn.rearrange("(j p) m -> p j m", p=128)
            with (
                tc.tile_pool(name="znp", bufs=14) as znp,
                tc.tile_pool(name="csp", bufs=1) as csp,
            ):
                # zn rides the sync ring (HWDGE; idle once the zT tails
                # drain at ~45us — the gpsimd SWDGE issues were hiccuping
                # the PE ~0.4us per slot reuse).  A WAR-gated dma_start
                # blocks its issuing ENGINE, which is safe here: sync's next
                # work (out writes, ~169us) comes after the last zn slot
                # frees (~155us).  Wv (needed only at ~170us) rides gpsimd
                # at S-start, off the critical load window.
                zn_sb = []
                for j in range(14):
                    zn_t = znp.tile([128, D], F16, tag="zn", name=f"zn{j}")
                    nc.sync.dma_start(zn_t[:], znv[:, j, :])
                    zn_sb.append(zn_t)
                for t in range(DT):
                    nc.gpsimd.dma_start(wv_sb[:, t, :], wvv[:, t, :])
                for j in range(14, JT):
                    zn_t = znp.tile([128, D], F16, tag="zn", name=f"zn{j}")
                    nc.sync.dma_start(zn_t[:], znv[:, j, :])
                    zn_sb.append(zn_t)

                mult_sb = misc.tile([128, ST], F32)
                with (
                    tc.tile_pool(name="ps_s", bufs=2, space="PSUM") as ps_s,
                    tc.tile_pool(name="ps_rs", bufs=1, space="PSUM") as ps_rs,
                ):
                    rs_ps = ps_rs.tile([128, ROWS], F32)
                    # rowsum for j is issued AFTER S j+1's matmuls: it waits
                    # on ACT's exp(j), which then overlaps S j+1 on the PE
                    def rowsum(j):
                        nc.tensor.matmul(rs_ps[:], ones128[:], expS[:, j, :],
                                         start=(j == 0), stop=(j == JT - 1))

                    for j in range(JT):
                        ps_S = ps_s.tile([128, ROWS], F32, tag="pss")
                        for t in range(DT):
                            nc.tensor.matmul(
                                ps_S[:], zt_sb[:, t, j * 128:(j + 1) * 128],
                                B_sb[:, t, :],
                                start=(t == 0), stop=(t == DT - 1))
                        nc.scalar.activation(expS[:, j, :], ps_S[:], Exp,
                                             bias=bias_sb[:], scale=1.0)
                        if j > 0:
                            rowsum(j - 1)
                    rowsum(JT - 1)

                    # row-sum -> per-row reciprocal multipliers [128, ST]
                    rs_sb = misc.tile([1, ROWS], F32)
                    nc.vector.tensor_copy(rs_sb[:], rs_ps[0:1, :])
                    rs_dram = dram.tile([1, ROWS], F32)
                    nc.scalar.dma_start(rs_dram[:], rs_sb[:])
                    rs128 = misc.tile([128, ST], F32)
                    nc.scalar.dma_start(
                        rs128[:], rs_dram[0, :].rearrange("(r p) -> p r",
                                                          p=128))
                    nc.vector.reciprocal(mult_sb[:], rs128[:])
                    nc.vector.tensor_scalar_mul(mult_sb[:], mult_sb[:], SCALE)

                # ---------------- C phase ---------------------------------
                # C^T[d, r] = sum_j zn[j, d-slice]^T P^T[j, r]
                C_sb = csp.tile([128, MT, ROWS], BF16)
                with tc.tile_pool(name="ps_c", bufs=8, space="PSUM") as ps_c:
                    psc = [ps_c.tile([128, ROWS], F32, tag="psc",
                                     name=f"psc{m}") for m in range(MT)]
                    for j in range(JT):
                        for m in range(MT):
                            nc.tensor.matmul(
                                psc[m][:], zn_sb[j][:, m * 128:(m + 1) * 128],
                                expS[:, j, :],
                                start=(j == 0), stop=(j == JT - 1))
                    for m in range(MT):
                        eng = (nc.vector.tensor_copy if m % 2 == 0
                               else nc.scalar.copy)
                        eng(C_sb[:, m, :], psc[m][:])

            # ---------------- out phase -----------------------------------
            # out[r-slice, e] = sum_m C^T[m, r-slice]^T Wv[m, e]
            with tc.tile_pool(name="ps_o", bufs=8, space="PSUM") as ps_o:
                for r in range(ST):
                    for h in range(2):
                        po = ps_o.tile([128, 512], F32, tag="po",
                                       name=f"po{r}{h}")
                        for m in range(MT):
                            nc.tensor.matmul(
                                po[:], C_sb[:, m, r * 128:(r + 1) * 128],
                                wv_sb[:, m, h * 512:(h + 1) * 512],
                                start=(m == 0), stop=(m == MT - 1))
                        o_sb = outp.tile([128, 512], F32, tag="osb")
                        nc.vector.tensor_scalar_mul(o_sb[:], po[:],
                                                    mult_sb[:, r:r + 1])
                        # alternate rings so the final writes drain in
                        # parallel instead of serializing the tail
                        deng = nc.sync if (r * 2 + h) % 2 == 0 else nc.scalar
                        deng.dma_start(
                            d_out[r * 128:(r + 1) * 128,
                                  h * 512:(h + 1) * 512],
                            o_sb[:])
    nc.compile()
    return nc


_BUILT = None


def make_in_maps(z, Wq, Wk, Wv):
    zT = np.ascontiguousarray(z.T).astype(np.float16)
    zn = z.astype(np.float16)
    wq16 = Wq.astype(np.float16)
    wkt16 = np.ascontiguousarray(Wk.T).astype(np.float16)
    wv16 = Wv.astype(np.float16)
    in_maps = []
    for c in range(NCORES):
        in_maps.append({
            "zT": np.ascontiguousarray(np.roll(zT, -c * ROWS, axis=1)),
            "zn": np.ascontiguousarray(np.roll(zn, -c * ROWS, axis=0)),
            "Wq": wq16,
            "WkT": wkt16,
            "Wv": wv16,
        })
    return in_maps


def kernel(z, Wq, Wk, Wv):
    global _BUILT
    from concourse.bass_utils import run_bass_kernel_spmd

    if _BUILT is None:
        _BUILT = _build()
    nc = _BUILT

    in_maps = make_in_maps(z, Wq, Wk, Wv)
    res = run_bass_kernel_spmd(nc, in_maps, list(range(NCORES)))
    out = np.concatenate([res.results[c]["out"] for c in range(NCORES)], axis=0)
    return out.astype(np.float32)


if __name__ == "__main__":
    rng = np.random.default_rng(0)
    z = rng.standard_normal((SEQ, D)).astype(np.float32)
    Wq = (0.02 * rng.standard_normal((D, DK))).astype(np.float32)
    Wk = (0.02 * rng.standard_normal((D, DK))).astype(np.float32)
    Wv = (0.02 * rng.standard_normal((D, DV))).astype(np.float32)
    out = kernel(z=z, Wq=Wq, Wk=Wk, Wv=Wv)
    print(out.shape, out.dtype)


# revision 30
# speedup vs baseline: 1.0082x; 1.0082x over previous
"""Distributed single-head attention on 8 TRN2 NeuronCores — zero-collective.

Math (matches the reference):
    q = z @ Wq; k = z @ Wk; v = z @ Wv
    out = softmax(q k^T) * DK**-0.5 @ v

Key idea: every core receives the FULL z (inputs are full-size anyway), so
k and v never need to be materialized or all-gathered.  Using associativity:
    S_r  = q_r k^T = (z_r Wq) Wk^T z^T      -> B^T = Wk q_r^T, S^T = z B^T
    out_r = P_r v  = (P_r z) Wv             -> C^T = z^T-accum of P^T, out = C^T^T Wv
Per-core FLOPs are identical to the gather-based flash schedule (736 unit
matmuls), but there are NO collectives: no skew-absorbing barrier, no
serialized CC stream, no PE stall waiting for gathered K/V (the baseline
idled the PE 38us there, which also re-throttled the HAM clock gate).

Sharding: core c processes rows [512c, 512c+512).  Host ships z rolled so
each core's own block is first: zT_roll (d-major, for S) and zn_roll
(seq-major, for C).  A j-tile index in the kernel is the global row
(c*512 + 128j) mod 4096 — a pure permutation, harmless under the j-sums.

Phases (all PE-dense, back-to-back):
    q^T (64 MM) -> B^T (64) -> S^T/exp/rowsum (256+32) -> C^T (256) -> out (64)
DMA: ~22MB of params per core against ~255-358GB/s aggregate read BW, so
strict global need order: (wq scalar || zT sync) for q, WkT then zT tails
on sync, zn prefetch + Wv only from S-phase start.  PSUM: 8 banks for
projections, 2+1 for S/rowsum, 8 for C^T, 8 for out — sequential scopes.

Precision: fp16 z/W/q/B + f32 PSUM keeps logits to ~6e-3 abs err; exp and
C^T in bf16 (range: logits can reach ~70 pre-shift, so exp(S-40) can hit
e^30 — fp16 would overflow, bf16 is safe).  End-to-end rel err ~3e-3.
"""

import numpy as np

SEQ, D, DK, DV = 4096, 1024, 1024, 1024
NCORES = 8
ROWS = SEQ // NCORES            # 512 rows per core
DT = D // 128                   # 8 contraction tiles (input dim)
MT = DK // 128                  # 8 dk tiles
ST = ROWS // 128                # 4 local seq tiles
JT = SEQ // 128                 # 32 global seq tiles
SHIFT = 40.0                    # constant logit shift (softmax-invariant)
SCALE = DK ** -0.5


def _build():
    import concourse.mybir as mybir
    import concourse.tile as tile
    from concourse import bacc

    F32 = mybir.dt.float32
    F16 = mybir.dt.float16
    BF16 = mybir.dt.bfloat16
    Exp = mybir.ActivationFunctionType.Exp

    nc = bacc.Bacc("TRN2", target_bir_lowering=False, debug=False, num_devices=NCORES)
    d_zT = nc.declare_dram_parameter("zT", [D, SEQ], F16, isOutput=False)
    d_zn = nc.declare_dram_parameter("zn", [SEQ, D], F16, isOutput=False)
    d_wq = nc.declare_dram_parameter("Wq", [D, DK], F16, isOutput=False)
    d_wkt = nc.declare_dram_parameter("WkT", [DK, D], F16, isOutput=False)
    d_wv = nc.declare_dram_parameter("Wv", [D, DV], F16, isOutput=False)
    d_out = nc.declare_dram_parameter("out", [ROWS, DV], F32, isOutput=True)

    with tile.TileContext(nc) as tc:
        with (
            tc.tile_pool(name="dram", bufs=1, space="DRAM") as dram,
            tc.tile_pool(name="misc", bufs=1) as misc,
            tc.tile_pool(name="zt", bufs=1) as ztp,
            tc.tile_pool(name="expp", bufs=1) as expp,
            tc.tile_pool(name="wvp", bufs=1) as wvp,
            tc.tile_pool(name="qb", bufs=1) as qbp,
            tc.tile_pool(name="outp", bufs=4) as outp,
        ):
            # constants: full-width ones for the PE rowsum, exp bias
            ones128 = misc.tile([128, 128], BF16)
            nc.vector.memset(ones128[:], 1.0)
            bias_sb = misc.tile([128, 1], F32)
            nc.vector.memset(bias_sb[:], -SHIFT)
            warm_sb = misc.tile([128, 1], F32)

            # ---- resident loads ------------------------------------------
            # sync ring, strict need order: (wq_t, zT jchunk0_t) pairs so the
            # t-outer q projection starts after one pair; then WkT for B^T;
            # then the rest of zT for the S phase.
            ztv = d_zT.rearrange("(t p) n -> p t n", p=128)
            wqv = d_wq.rearrange("(t p) m -> p t m", p=128)
            wktv = d_wkt.rearrange("(t p) m -> p t m", p=128)
            wvv = d_wv.rearrange("(t p) m -> p t m", p=128)

            zt_sb = ztp.tile([128, DT, SEQ], F16)
            # Wv loads late (gpsimd ring, at S-phase start): per-core
            # AGGREGATE DMA read BW is only ~255GB/s across all rings, so
            # anything loading during the projection phase steals bandwidth
            # from the critical (wq, zT, WkT) stream
            wv_sb = wvp.tile([128, DT, DV], F16)

            B_sb = qbp.tile([128, MT, ROWS], F16)
            expS = expp.tile([128, JT, ROWS], BF16)

            with (
                tc.tile_pool(name="wqk", bufs=1) as wqk,
                tc.tile_pool(name="ps_proj", bufs=8, space="PSUM") as psp,
            ):
                # q-phase feed: while both rings pull, each gets only
                # ~half the ~255-358GB/s per-core aggregate — less than the
                # q pass's per-ring appetite if wq rides one ring alone.
                # So wq interleaves across BOTH rings by t-parity, the sync
                # ring carries only the [0:512] zT halves q actually reads,
                # and everything q/B doesn't need yet (zT [512:1024] halves,
                # jchunk1-3) loads after WkT.  Every operand then arrives
                # >=1.5us before its consuming matmul group.
                wq_sb = [wqk.tile([128, DK], F16, name=f"wq{t}")
                         for t in range(DT)]
                wkt_sb = [wqk.tile([128, D], F16, name=f"wkt{t}")
                          for t in range(DT)]
                # 128B dummy first: absorbs the scalar DMA queue's one-time
                # init latency (~4us observed on its first real transfer)
                qwarm = wqk.tile([1, 64], F16, name="qwarm")
                nc.scalar.dma_start(qwarm[:], wqv[0:1, 0, 0:64])
                # scalar ring: even-t wq (first one split so MM#0 starts asap)
                nc.scalar.dma_start(wq_sb[0][:, 0:128], wqv[:, 0, 0:128])
                nc.scalar.dma_start(wq_sb[0][:, 128:DK], wqv[:, 0, 128:DK])
                for t in (2, 4, 6):
                    nc.scalar.dma_start(wq_sb[t][:], wqv[:, t, :])
                # sync ring: zT own-halves with odd-t wq woven in need order
                for t in range(DT):
                    nc.sync.dma_start(zt_sb[:, t, 0:512], ztv[:, t, 0:512])
                    if t % 2 == 0:
                        nc.sync.dma_start(wq_sb[t + 1][:], wqv[:, t + 1, :])
                for t in range(DT):
                    nc.sync.dma_start(wkt_sb[t][:], wktv[:, t, :])
                for t in range(DT):
                    nc.sync.dma_start(zt_sb[:, t, 512:1024],
                                      ztv[:, t, 512:1024])
                # touch Exp only AFTER the scalar ring's wq issues: the
                # 1.3us ACT table load was delaying the first q operands;
                # it just needs to finish before the first real exp (~46us)
                nc.scalar.activation(warm_sb[:], bias_sb[:], Exp,
                                     bias=bias_sb[:], scale=1.0)
                # zT tails at per-jchunk granularity so S j>=8 never waits
                # on bytes it doesn't need yet
                for c in range(1, 4):
                    for t in range(DT):
                        nc.sync.dma_start(zt_sb[:, t, c * 1024:(c + 1) * 1024],
                                          ztv[:, t, c * 1024:(c + 1) * 1024])

                # q^T[dk, r] = sum_t Wq[t-rows, dk-slice]^T zT[t-rows, own r]
                q_sb = wqk.tile([128, MT, ROWS], F16, name="qsb")
                psq = [psp.tile([128, ROWS], F32, tag="psp", name=f"psq{m}")
                       for m in range(MT)]
                for t in range(DT):
                    for m in range(MT):
                        nc.tensor.matmul(
                            psq[m][:], wq_sb[t][:, m * 128:(m + 1) * 128],
                            zt_sb[:, t, 0:ROWS],
                            start=(t == 0), stop=(t == DT - 1))
                        if t == DT - 1:
                            # copy each bank the moment it stops, alternating
                            # engines, so bank reuse in B never waits a burst
                            eng = (nc.vector.tensor_copy if m % 2 == 0
                                   else nc.scalar.copy)
                            eng(q_sb[:, m, :], psq[m][:])

                # B^T[d, r] = sum_dk Wk[d-slice, dk]^T q^T -> lhsT = WkT tiles
                # t-outer so group t only needs wkt_t (arriving just-in-time
                # behind the zT jchunk0 stream)
                psb = [psp.tile([128, ROWS], F32, tag="psp", name=f"psb{m}")
                       for m in range(MT)]
                for t in range(MT):
                    for m in range(DT):
                        nc.tensor.matmul(
                            psb[m][:], wkt_sb[t][:, m * 128:(m + 1) * 128],
                            q_sb[:, t, :],
                            start=(t == 0), stop=(t == MT - 1))
                        if t == MT - 1:
                            eng = (nc.vector.tensor_copy if m % 2 == 0
                                   else nc.scalar.copy)
                            eng(B_sb[:, m, :], psb[m][:])

            # ---------------- S phase -------------------------------------
            # S^T[j, r] = sum_t zT[t, j-slice]^T B^T[t, r]; exp on ACT with
            # the -SHIFT bias; rowsum via ones-matmul accumulated across all
            # j into one persistent PSUM bank.
            znv = d_zn.rearrange("(j p) m -> p j m", p=128)
            with (
                tc.tile_pool(name="znp", bufs=14) as znp,
                tc.tile_pool(name="csp", bufs=1) as csp,
            ):
                # zn rides the sync ring (HWDGE; idle once the zT tails
                # drain at ~45us — the gpsimd SWDGE issues were hiccuping
                # the PE ~0.4us per slot reuse).  A WAR-gated dma_start
                # blocks its issuing ENGINE, which is safe here: sync's next
                # work (out writes, ~169us) comes after the last zn slot
                # frees (~155us).  Wv (needed only at ~170us) rides gpsimd
                # at S-start, off the critical load window.
                zn_sb = []
                for j in range(14):
                    zn_t = znp.tile([128, D], F16, tag="zn", name=f"zn{j}")
                    nc.sync.dma_start(zn_t[:], znv[:, j, :])
                    zn_sb.append(zn_t)
                for t in range(DT):
                    nc.gpsimd.dma_start(wv_sb[:, t, :], wvv[:, t, :])
                for j in range(14, JT):
                    zn_t = znp.tile([128, D], F16, tag="zn", name=f"zn{j}")
                    nc.sync.dma_start(zn_t[:], znv[:, j, :])
                    zn_sb.append(zn_t)

                mult_sb = misc.tile([128, ST], F32)
                with (
                    tc.tile_pool(name="ps_s", bufs=2, space="PSUM") as ps_s,
                    tc.tile_pool(name="ps_rs", bufs=1, space="PSUM") as ps_rs,
                ):
                    rs_ps = ps_rs.tile([128, ROWS], F32)
                    # rowsum for j is issued AFTER S j+1's matmuls: it waits
                    # on ACT's exp(j), which then overlaps S j+1 on the PE
                    def rowsum(j):
                        nc.tensor.matmul(rs_ps[:], ones128[:], expS[:, j, :],
                                         start=(j == 0), stop=(j == JT - 1))

                    for j in range(JT):
                        ps_S = ps_s.tile([128, ROWS], F32, tag="pss")
                        for t in range(DT):
                            nc.tensor.matmul(
                                ps_S[:], zt_sb[:, t, j * 128:(j + 1) * 128],
                                B_sb[:, t, :],
                                start=(t == 0), stop=(t == DT - 1))
                        nc.scalar.activation(expS[:, j, :], ps_S[:], Exp,
                                             bias=bias_sb[:], scale=1.0)
                        if j > 0:
                            rowsum(j - 1)
                    rowsum(JT - 1)

                    # row-sum -> per-row reciprocal multipliers [128, ST]
                    rs_sb = misc.tile([1, ROWS], F32)
                    nc.vector.tensor_copy(rs_sb[:], rs_ps[0:1, :])
                    rs_dram = dram.tile([1, ROWS], F32)
                    nc.scalar.dma_start(rs_dram[:], rs_sb[:])
                    rs128 = misc.tile([128, ST], F32)
                    nc.scalar.dma_start(
                        rs128[:], rs_dram[0, :].rearrange("(r p) -> p r",
                                                          p=128))
                    nc.vector.reciprocal(mult_sb[:], rs128[:])
                    nc.vector.tensor_scalar_mul(mult_sb[:], mult_sb[:], SCALE)

                # ---------------- C phase ---------------------------------
                # C^T[d, r] = sum_j zn[j, d-slice]^T P^T[j, r]
                C_sb = csp.tile([128, MT, ROWS], BF16)
                with tc.tile_pool(name="ps_c", bufs=8, space="PSUM") as ps_c:
                    psc = [ps_c.tile([128, ROWS], F32, tag="psc",
                                     name=f"psc{m}") for m in range(MT)]
                    for j in range(JT):
                        for m in range(MT):
                            nc.tensor.matmul(
                                psc[m][:], zn_sb[j][:, m * 128:(m + 1) * 128],
                                expS[:, j, :],
                                start=(j == 0), stop=(j == JT - 1))
                    for m in range(MT):
                        eng = (nc.vector.tensor_copy if m % 2 == 0
                               else nc.scalar.copy)
                        eng(C_sb[:, m, :], psc[m][:])

            # ---------------- out phase -----------------------------------
            # out[r-slice, e] = sum_m C^T[m, r-slice]^T Wv[m, e]
            with tc.tile_pool(name="ps_o", bufs=8, space="PSUM") as ps_o:
                for r in range(ST):
                    for h in range(2):
                        po = ps_o.tile([128, 512], F32, tag="po",
                                       name=f"po{r}{h}")
                        for m in range(MT):
                            nc.tensor.matmul(
                                po[:], C_sb[:, m, r * 128:(r + 1) * 128],
                                wv_sb[:, m, h * 512:(h + 1) * 512],
                                start=(m == 0), stop=(m == MT - 1))
                        o_sb = outp.tile([128, 512], F32, tag="osb")
                        nc.vector.tensor_scalar_mul(o_sb[:], po[:],
                                                    mult_sb[:, r:r + 1])
                        # alternate rings so the final writes drain in
                        # parallel instead of serializing the tail
                        deng = nc.sync if (r * 2 + h) % 2 == 0 else nc.scalar
                        deng.dma_start(
                            d_out[r * 128:(r + 1) * 128,
                                  h * 512:(h + 1) * 512],
                            o_sb[:])
    nc.compile()
    return nc


_BUILT = None


def make_in_maps(z, Wq, Wk, Wv):
    zT = np.ascontiguousarray(z.T).astype(np.float16)
    zn = z.astype(np.float16)
    wq16 = Wq.astype(np.float16)
    wkt16 = np.ascontiguousarray(Wk.T).astype(np.float16)
    wv16 = Wv.astype(np.float16)
    in_maps = []
    for c in range(NCORES):
        in_maps.append({
            "zT": np.ascontiguousarray(np.roll(zT, -c * ROWS, axis=1)),
            "zn": np.ascontiguousarray(np.roll(zn, -c * ROWS, axis=0)),
            "Wq": wq16,
            "WkT": wkt16,
            "Wv": wv16,
        })
    return in_maps


def kernel(z, Wq, Wk, Wv):
    global _BUILT
    from concourse.bass_utils import run_bass_kernel_spmd

    if _BUILT is None:
        _BUILT = _build()
    nc = _BUILT

    in_maps = make_in_maps(z, Wq, Wk, Wv)
    res = run_bass_kernel_spmd(nc, in_maps, list(range(NCORES)))
    out = np.concatenate([res.results[c]["out"] for c in range(NCORES)], axis=0)
    return out.astype(np.float32)


if __name__ == "__main__":
    rng = np.random.default_rng(0)
    z = rng.standard_normal((SEQ, D)).astype(np.float32)
    Wq = (0.02 * rng.standard_normal((D, DK))).astype(np.float32)
    Wk = (0.02 * rng.standard_normal((D, DK))).astype(np.float32)
    Wv = (0.02 * rng.standard_normal((D, DV))).astype(np.float32)
    out = kernel(z=z, Wq=Wq, Wk=Wk, Wv=Wv)
    print(out.shape, out.dtype)
